# revision 6
# baseline (speedup 1.0000x reference)
"""Trainium2 Bass kernel for the CPS-TCN model.

Model: embedding gather -> 4 TCN levels (2 dilated causal convs, K=2,
dilations 1,2,4,8, relu + residual) -> linear decoder [C=512 -> OUT=11]
-> multiplicative mask.  B=32, L=1024, C=512, VOCAB=128.

Sharding: data-parallel over B across 8 NeuronCores (4 sequences/core),
weights replicated.

Per-core implementation notes:
- The embedding gather is computed on the PE as emb.T @ onehot(x); the
  one-hot is built on-chip (iota + is_equal against a PE-broadcast of x).
- Conv taps are [128,128] x [128,512] matmuls accumulated in PSUM; the
  dilation shift is a free-dim slice into a left-zero-padded activation
  buffer (pad=8 >= max dilation).
- Level-0 conv1 contracts over the 128-entry vocab directly using
  host-precomputed effective weights (emb @ w1[0].T), saving 3/4 of the
  contraction chunks for that conv.
- All matmul operands are float32r (TF32-like single-pass PE mode,
  fp32 accumulate in PSUM).
- Levels >= 1 skip the outer residual relu: both summands are already
  non-negative.
"""

import os
import sys
import types

sys.path.insert(0, "/opt/trn_rl_repo")

import numpy as np

B, L, C, K, NLEV, VOCAB, OUT = 32, 1024, 512, 2, 4, 128, 11
N_CORES = 8
B_SHARD = B // N_CORES          # 4 sequences per core
SEQ_PER_PASS = 2                # process 2 sequences per level sweep
N_PASS = B_SHARD // SEQ_PER_PASS
PAD = 8                         # left zero pad >= max dilation
CCH = C // 128                  # 4 channel chunks of 128
LCH = L // 512                  # 2 free-dim chunks of 512
CW = PAD + L                    # per-chunk width in the activation layout


def _install_trace_shim():
    """Register the axon NTFF profiling hook if tracing is requested.

    Only needed when BASS_TRACE=1; without it run_bass_kernel_spmd would
    crash importing the (absent) antenv.axon_hooks module.
    """
    import antenv

    if "antenv.axon_hooks" in sys.modules:
        return
    hooks_mod = types.ModuleType("antenv.axon_hooks")
    hooks_mod._hook = None

    def set_axon_ntff_profile_hook(h):
        hooks_mod._hook = h

    def get_axon_ntff_profile_hook():
        return hooks_mod._hook

    hooks_mod.set_axon_ntff_profile_hook = set_axon_ntff_profile_hook
    hooks_mod.get_axon_ntff_profile_hook = get_axon_ntff_profile_hook
    sys.modules["antenv.axon_hooks"] = hooks_mod
    antenv.axon_hooks = hooks_mod
    try:
        from trn_agent_boot.trn_boot import _ntff_profile_via_ctypes

        hook = _ntff_profile_via_ctypes("/opt/axon/libaxon_pjrt.so")
        set_axon_ntff_profile_hook(hook)
    except Exception:
        pass

    import concourse.bass_utils as bass_utils_mod

    bass_utils_mod.upload_artifacts = lambda tmpdir: "/tmp/no-upload"


_CACHED = {}


def _build():
    from concourse import bacc
    import concourse.mybir as mybir
    import concourse.tile as tile

    F32 = mybir.dt.float32
    F32R = mybir.dt.float32r
    AF = mybir.ActivationFunctionType
    ALU = mybir.AluOpType

    nc = bacc.Bacc("TRN2")

    xf_d = nc.dram_tensor("xf", [B_SHARD, L], F32R, kind="ExternalInput")
    mkf_d = nc.dram_tensor("mkf", [B_SHARD, L], F32R, kind="ExternalInput")
    emb_d = nc.dram_tensor("emb", [VOCAB, C], F32R, kind="ExternalInput")
    w1e_d = nc.dram_tensor("w1e", [K, VOCAB, C], F32R, kind="ExternalInput")
    w1t_d = nc.dram_tensor("w1t", [NLEV - 1, K, CCH, 128, C], F32R, kind="ExternalInput")
    w2t_d = nc.dram_tensor("w2t", [NLEV, K, CCH, 128, C], F32R, kind="ExternalInput")
    b1_d = nc.dram_tensor("b1c", [128, NLEV * CCH], F32, kind="ExternalInput")
    b2_d = nc.dram_tensor("b2c", [128, NLEV * CCH], F32, kind="ExternalInput")
    dec_d = nc.dram_tensor("decT", [CCH, 128, OUT], F32R, kind="ExternalInput")
    db_d = nc.dram_tensor("decb", [OUT, 1], F32, kind="ExternalInput")
    y_d = nc.dram_tensor("y", [B_SHARD, OUT, L], F32, kind="ExternalOutput")

    def chunk(off_cc, lc, shift=0):
        # free-dim slice for channel chunk `off_cc`, L-chunk `lc`, shifted
        # left by `shift` (reads into the zero pad for leading positions)
        s = off_cc * CW + PAD + lc * 512 - shift
        return slice(s, s + 512)

    with tile.TileContext(nc) as tc:
        with tc.tile_pool(name="const", bufs=1) as cpool, \
             tc.tile_pool(name="wpool", bufs=2) as wpool, \
             tc.tile_pool(name="acts", bufs=2) as apool, \
             tc.tile_pool(name="psum", bufs=8, space="PSUM") as pp:

            # ---- constants ----
            emb = cpool.tile([VOCAB, C], F32R)
            nc.sync.dma_start(out=emb[:], in_=emb_d[:])
            w1e = []
            for k in range(K):
                w1ek = cpool.tile([VOCAB, C], F32R, name=f"w1e{k}")
                nc.sync.dma_start(out=w1ek[:], in_=w1e_d[k])
                w1e.append(w1ek)
            b1c = cpool.tile([128, NLEV * CCH], F32)
            nc.sync.dma_start(out=b1c[:], in_=b1_d[:])
            b2c = cpool.tile([128, NLEV * CCH], F32)
            nc.sync.dma_start(out=b2c[:], in_=b2_d[:])
            decT = []
            for ci in range(CCH):
                dct = cpool.tile([128, OUT], F32R, name=f"decT{ci}")
                nc.sync.dma_start(out=dct[:], in_=dec_d[ci])
                decT.append(dct)
            decb = cpool.tile([OUT, 1], F32)
            nc.sync.dma_start(out=decb[:], in_=db_d[:])

            ones_f = cpool.tile([1, 128], F32)
            nc.vector.memset(ones_f[:], 1.0)
            ones = cpool.tile([1, 128], F32R)
            nc.vector.tensor_copy(ones[:], ones_f[:])
            iota = cpool.tile([128, 1], F32)
            nc.gpsimd.iota(iota[:], pattern=[[0, 1]], base=0,
                           channel_multiplier=1,
                           allow_small_or_imprecise_dtypes=True)
            zpad = cpool.tile([128, PAD], F32)
            nc.vector.memset(zpad[:], 0.0)

            def zero_pads(t, nchunk=CCH):
                for cc in range(nchunk):
                    nc.vector.tensor_copy(t[:, cc * CW:cc * CW + PAD], zpad[:])

            for p in range(N_PASS):
                seqs = [p * SEQ_PER_PASS + s for s in range(SEQ_PER_PASS)]
                # hs[b] = current level-input state tile for sequence b
                hs = {}

                # ---- level 0: build one-hot, embedding, first level ----
                lev = 0
                d = 1
                w2_t = {}
                for k in range(K):
                    for ci in range(CCH):
                        w2_t[k, ci] = wpool.tile(
                            [128, C], F32R, tag=f"w2_{k}_{ci}", name=f"w2_{k}_{ci}")
                        nc.sync.dma_start(out=w2_t[k, ci][:],
                                          in_=w2t_d[lev, k, ci])

                ohs = {}
                for b in seqs:
                    xf = apool.tile([1, L], F32R, tag="xf", name=f"xf{b}")
                    nc.sync.dma_start(out=xf[:], in_=xf_d[b:b + 1, :])
                    oh = apool.tile([128, CW], F32R, tag="oh", name=f"oh{b}")
                    zero_pads(oh, 1)
                    for lc in range(LCH):
                        ps = pp.tile([128, 512], F32, tag="ps", name=f"psb{b}_{lc}")
                        nc.tensor.matmul(ps[:], ones[:],
                                         xf[:, lc * 512:(lc + 1) * 512],
                                         start=True, stop=True)
                        nc.vector.tensor_scalar(
                            out=oh[:, PAD + lc * 512:PAD + (lc + 1) * 512],
                            in0=ps[:], scalar1=iota[:], scalar2=None,
                            op0=ALU.is_equal)
                    ohs[b] = oh

                    # h0 = emb.T @ onehot  (residual input of level 0)
                    h0 = apool.tile([128, CCH * CW], F32R, tag="hs", bufs=3,
                                    name=f"h0_{b}")
                    zero_pads(h0)
                    for cc in range(CCH):
                        for lc in range(LCH):
                            ps = pp.tile([128, 512], F32, tag="ps", name=f"pse{b}_{cc}_{lc}")
                            nc.tensor.matmul(
                                ps[:], emb[:, cc * 128:(cc + 1) * 128],
                                oh[:, chunk(0, lc)], start=True, stop=True)
                            nc.vector.tensor_copy(h0[:, chunk(cc, lc)], ps[:])
                    hs[b] = h0

                for b in seqs:
                    oh = ohs[b]
                    # conv1 of level 0: contract over vocab with w1e
                    h1 = apool.tile([128, CCH * CW], F32R, tag="h1",
                                    name=f"h1_{b}_{lev}")
                    zero_pads(h1)
                    for cc in range(CCH):
                        for lc in range(LCH):
                            ps = pp.tile([128, 512], F32, tag="ps",
                                         name=f"ps1_{b}_{cc}_{lc}")
                            for k in range(K):
                                nc.tensor.matmul(
                                    ps[:],
                                    w1e[k][:, cc * 128:(cc + 1) * 128],
                                    oh[:, chunk(0, lc, (K - 1 - k) * d)],
                                    start=(k == 0), stop=(k == K - 1))
                            nc.scalar.activation(
                                h1[:, chunk(cc, lc)], ps[:], AF.Relu,
                                bias=b1c[:, lev * CCH + cc:lev * CCH + cc + 1],
                                scale=1.0)
                    _conv2_and_res(nc, tc, pp, apool, hs, b, lev, d, h1,
                                   w2_t, b2c, zero_pads, chunk, mybir)

                # ---- levels 1..3 ----
                for lev in range(1, NLEV):
                    d = 2 ** lev
                    w1_t = {}
                    for k in range(K):
                        for ci in range(CCH):
                            w1_t[k, ci] = wpool.tile(
                                [128, C], F32R, tag=f"w1_{k}_{ci}",
                                name=f"w1_{k}_{ci}_{lev}")
                            nc.sync.dma_start(out=w1_t[k, ci][:],
                                              in_=w1t_d[lev - 1, k, ci])
                    w2_t = {}
                    for k in range(K):
                        for ci in range(CCH):
                            w2_t[k, ci] = wpool.tile(
                                [128, C], F32R, tag=f"w2_{k}_{ci}",
                                name=f"w2_{k}_{ci}_{lev}")
                            nc.sync.dma_start(out=w2_t[k, ci][:],
                                              in_=w2t_d[lev, k, ci])

                    for b in seqs:
                        h = hs[b]
                        h1 = apool.tile([128, CCH * CW], F32R, tag="h1",
                                        name=f"h1_{b}_{lev}")
                        zero_pads(h1)
                        for cc in range(CCH):
                            for lc in range(LCH):
                                ps = pp.tile([128, 512], F32, tag="ps",
                                             name=f"ps1_{b}_{cc}_{lc}_{lev}")
                                first = True
                                for k in range(K):
                                    for ci in range(CCH):
                                        nc.tensor.matmul(
                                            ps[:],
                                            w1_t[k, ci][:, cc * 128:(cc + 1) * 128],
                                            h[:, chunk(ci, lc, (K - 1 - k) * d)],
                                            start=first,
                                            stop=(k == K - 1 and ci == CCH - 1))
                                        first = False
                                nc.scalar.activation(
                                    h1[:, chunk(cc, lc)], ps[:], AF.Relu,
                                    bias=b1c[:, lev * CCH + cc:lev * CCH + cc + 1],
                                    scale=1.0)
                        _conv2_and_res(nc, tc, pp, apool, hs, b, lev, d, h1,
                                       w2_t, b2c, zero_pads, chunk, mybir)

                # ---- decoder + mask ----
                for b in seqs:
                    h = hs[b]
                    mk = apool.tile([1, L], F32R, tag="mk", name=f"mk{b}")
                    nc.sync.dma_start(out=mk[:], in_=mkf_d[b:b + 1, :])
                    msk = apool.tile([OUT, L], F32, tag="msk", name=f"msk{b}")
                    ym = apool.tile([OUT, L], F32, tag="ym", name=f"ym{b}")
                    for lc in range(LCH):
                        psm = pp.tile([OUT, 512], F32, tag="ps", name=f"psm{b}_{lc}")
                        nc.tensor.matmul(psm[:], ones[:, 0:OUT],
                                         mk[:, lc * 512:(lc + 1) * 512],
                                         start=True, stop=True)
                        nc.vector.tensor_copy(
                            msk[:, lc * 512:(lc + 1) * 512], psm[:])
                        psd = pp.tile([OUT, 512], F32, tag="ps", name=f"psd{b}_{lc}")
                        for ci in range(CCH):
                            nc.tensor.matmul(
                                psd[:], decT[ci],
                                h[:, chunk(ci, lc)],
                                start=(ci == 0), stop=(ci == CCH - 1))
                        ysb = apool.tile([OUT, 512], F32, tag="ysb",
                                         name=f"ysb{b}_{lc}")
                        nc.scalar.activation(ysb[:], psd[:],
                                             AF.Identity,
                                             bias=decb[:], scale=1.0)
                        nc.vector.tensor_tensor(
                            out=ym[:, lc * 512:(lc + 1) * 512],
                            in0=ysb[:],
                            in1=msk[:, lc * 512:(lc + 1) * 512],
                            op=ALU.mult)
                    nc.sync.dma_start(out=y_d[b], in_=ym[:])

    nc.compile()
    return nc


def _conv2_and_res(nc, tc, pp, apool, hs, b, lev, d, h1, w2_t, b2c,
                   zero_pads, chunk, mybir):
    """conv2 of a level + residual; replaces hs[b] with the new state."""
    F32 = mybir.dt.float32
    F32R = mybir.dt.float32r
    AF = mybir.ActivationFunctionType
    ALU = mybir.AluOpType
    CCHl, LCHl = CCH, LCH
    h = hs[b]
    hn = apool.tile([128, CCH * CW], F32R, tag="hs", bufs=3,
                    name=f"hn_{b}_{lev}")
    zero_pads(hn)
    for cc in range(CCHl):
        for lc in range(LCHl):
            ps = pp.tile([128, 512], F32, tag="ps", name=f"ps2_{b}_{cc}_{lc}_{lev}")
            first = True
            for k in range(K):
                for ci in range(CCHl):
                    nc.tensor.matmul(
                        ps[:],
                        w2_t[k, ci][:, cc * 128:(cc + 1) * 128],
                        h1[:, chunk(ci, lc, (K - 1 - k) * d)],
                        start=first,
                        stop=(k == K - 1 and ci == CCHl - 1))
                    first = False
            h2 = apool.tile([128, 512], F32R, tag="h2",
                            name=f"h2_{b}_{cc}_{lc}_{lev}")
            nc.scalar.activation(
                h2[:], ps[:], AF.Relu,
                bias=b2c[:, lev * CCHl + cc:lev * CCHl + cc + 1], scale=1.0)
            if lev == 0:
                # h0 can be negative -> outer relu needed
                rt = apool.tile([128, 512], F32, tag="rtmp",
                                name=f"rt_{b}_{cc}_{lc}")
                nc.vector.tensor_tensor(out=rt[:], in0=h2[:],
                                        in1=h[:, chunk(cc, lc)], op=ALU.add)
                nc.scalar.activation(hn[:, chunk(cc, lc)], rt[:], AF.Relu)
            else:
                # both summands >= 0: outer relu is the identity
                nc.vector.tensor_tensor(out=hn[:, chunk(cc, lc)], in0=h2[:],
                                        in1=h[:, chunk(cc, lc)], op=ALU.add)
    hs[b] = hn


def _prep_inputs(x, mask, emb, w1, b1, w2, b2, dec_w, dec_b):
    """Host-side layout transforms; returns the per-core in_maps."""
    xf = x.astype(np.float32)
    mkf = mask.astype(np.float32)
    embf = np.asarray(emb, np.float32)
    w1f = np.asarray(w1, np.float32)
    w2f = np.asarray(w2, np.float32)
    # effective level-0 conv1 weights: contract over vocab instead of C
    # w1e[k][v, co] = sum_ci emb[v, ci] * w1[0][co, ci, k]
    w1e = np.stack([embf @ w1f[0, :, :, k].T for k in range(K)]).astype(np.float32)
    # [NLEV, K, Cin, Cout] chunked on Cin
    w1t = np.ascontiguousarray(w1f.transpose(0, 3, 2, 1))[1:].reshape(
        NLEV - 1, K, CCH, 128, C).astype(np.float32)
    w2t = np.ascontiguousarray(w2f.transpose(0, 3, 2, 1)).reshape(
        NLEV, K, CCH, 128, C).astype(np.float32)
    b1c = np.ascontiguousarray(
        np.asarray(b1, np.float32).reshape(NLEV, CCH, 128).transpose(2, 0, 1)
    ).reshape(128, NLEV * CCH)
    b2c = np.ascontiguousarray(
        np.asarray(b2, np.float32).reshape(NLEV, CCH, 128).transpose(2, 0, 1)
    ).reshape(128, NLEV * CCH)
    decT = np.ascontiguousarray(
        np.asarray(dec_w, np.float32).T.reshape(CCH, 128, OUT))
    decb = np.asarray(dec_b, np.float32).reshape(OUT, 1)

    in_maps = []
    for c in range(N_CORES):
        sl = slice(c * B_SHARD, (c + 1) * B_SHARD)
        in_maps.append(dict(
            xf=np.ascontiguousarray(xf[sl]),
            mkf=np.ascontiguousarray(mkf[sl]),
            emb=embf, w1e=w1e, w1t=w1t, w2t=w2t,
            b1c=b1c, b2c=b2c, decT=decT, decb=decb,
        ))
    return in_maps


def kernel(x, mask, emb, w1, b1, w2, b2, dec_w, dec_b):
    trace = bool(os.environ.get("BASS_TRACE"))
    if trace:
        _install_trace_shim()

    from concourse.bass_utils import run_bass_kernel_spmd

    if "nc" not in _CACHED:
        _CACHED["nc"] = _build()
    nc = _CACHED["nc"]

    in_maps = _prep_inputs(x, mask, emb, w1, b1, w2, b2, dec_w, dec_b)
    res = run_bass_kernel_spmd(nc, in_maps, list(range(N_CORES)),
                               trace=trace)
    _CACHED["last_result"] = res
    y = np.concatenate([res.results[c]["y"] for c in range(N_CORES)], axis=0)
    return np.ascontiguousarray(y.transpose(0, 2, 1))


# revision 8
# speedup vs baseline: 1.0186x; 1.0186x over previous
"""Trainium2 Bass kernel for the CPS-TCN model.

Model: embedding gather -> 4 TCN levels (2 dilated causal convs, K=2,
dilations 1,2,4,8, relu + residual) -> linear decoder [C=512 -> OUT=11]
-> multiplicative mask.  B=32, L=1024, C=512, VOCAB=128.

Sharding: data-parallel over B across 8 NeuronCores (4 sequences/core),
weights replicated.

Per-core implementation notes:
- The embedding gather is computed on the PE as emb.T @ onehot(x); the
  one-hot is built on-chip (iota + is_equal against a PE-broadcast of x).
- Conv taps are [128,128] x [128,512] matmuls accumulated in PSUM; the
  dilation shift is a free-dim slice into a left-zero-padded activation
  buffer (pad=8 >= max dilation).
- Level-0 conv1 contracts over the 128-entry vocab directly using
  host-precomputed effective weights (emb @ w1[0].T), saving 3/4 of the
  contraction chunks for that conv.
- All matmul operands are float32r (TF32-like single-pass PE mode,
  fp32 accumulate in PSUM).
- Levels >= 1 skip the outer residual relu: both summands are already
  non-negative.
"""

import os
import sys
import types

sys.path.insert(0, "/opt/trn_rl_repo")

import numpy as np

B, L, C, K, NLEV, VOCAB, OUT = 32, 1024, 512, 2, 4, 128, 11
N_CORES = 8
B_SHARD = B // N_CORES          # 4 sequences per core
SEQ_PER_PASS = 2                # process 2 sequences per level sweep
N_PASS = B_SHARD // SEQ_PER_PASS
PAD = 8                         # left zero pad >= max dilation
CCH = C // 128                  # 4 channel chunks of 128
LCH = L // 512                  # 2 free-dim chunks of 512
CW = PAD + L                    # per-chunk width in the activation layout


def _install_trace_shim():
    """Register the axon NTFF profiling hook if tracing is requested.

    Only needed when BASS_TRACE=1; without it run_bass_kernel_spmd would
    crash importing the (absent) antenv.axon_hooks module.
    """
    import antenv

    if "antenv.axon_hooks" in sys.modules:
        return
    hooks_mod = types.ModuleType("antenv.axon_hooks")
    hooks_mod._hook = None

    def set_axon_ntff_profile_hook(h):
        hooks_mod._hook = h

    def get_axon_ntff_profile_hook():
        return hooks_mod._hook

    hooks_mod.set_axon_ntff_profile_hook = set_axon_ntff_profile_hook
    hooks_mod.get_axon_ntff_profile_hook = get_axon_ntff_profile_hook
    sys.modules["antenv.axon_hooks"] = hooks_mod
    antenv.axon_hooks = hooks_mod
    try:
        from trn_agent_boot.trn_boot import _ntff_profile_via_ctypes

        hook = _ntff_profile_via_ctypes("/opt/axon/libaxon_pjrt.so")
        set_axon_ntff_profile_hook(hook)
    except Exception:
        pass

    import concourse.bass_utils as bass_utils_mod

    bass_utils_mod.upload_artifacts = lambda tmpdir: "/tmp/no-upload"


_CACHED = {}


def _build():
    from concourse import bacc
    import concourse.mybir as mybir
    import concourse.tile as tile

    F32 = mybir.dt.float32
    F32R = mybir.dt.float32r
    AF = mybir.ActivationFunctionType
    ALU = mybir.AluOpType

    nc = bacc.Bacc("TRN2")

    xf_d = nc.dram_tensor("xf", [B_SHARD, L], F32R, kind="ExternalInput")
    mkf_d = nc.dram_tensor("mkf", [B_SHARD, L], F32R, kind="ExternalInput")
    emb_d = nc.dram_tensor("emb", [VOCAB, C], F32R, kind="ExternalInput")
    w1e_d = nc.dram_tensor("w1e", [K, VOCAB, C], F32R, kind="ExternalInput")
    w1t_d = nc.dram_tensor("w1t", [NLEV - 1, K, CCH, 128, C], F32R, kind="ExternalInput")
    w2t_d = nc.dram_tensor("w2t", [NLEV, K, CCH, 128, C], F32R, kind="ExternalInput")
    b1_d = nc.dram_tensor("b1c", [128, NLEV * CCH], F32, kind="ExternalInput")
    b2_d = nc.dram_tensor("b2c", [128, NLEV * CCH], F32, kind="ExternalInput")
    dec_d = nc.dram_tensor("decT", [CCH, 128, OUT], F32R, kind="ExternalInput")
    db_d = nc.dram_tensor("decb", [OUT, 1], F32, kind="ExternalInput")
    y_d = nc.dram_tensor("y", [B_SHARD, OUT, L], F32, kind="ExternalOutput")

    def chunk(off_cc, lc, shift=0):
        # free-dim slice for channel chunk `off_cc`, L-chunk `lc`, shifted
        # left by `shift` (reads into the zero pad for leading positions)
        s = off_cc * CW + PAD + lc * 512 - shift
        return slice(s, s + 512)

    with tile.TileContext(nc) as tc:
        with tc.tile_pool(name="const", bufs=1) as cpool, \
             tc.tile_pool(name="wpool", bufs=2) as wpool, \
             tc.tile_pool(name="acts", bufs=2) as apool, \
             tc.tile_pool(name="psum", bufs=8, space="PSUM") as pp:

            # ---- constants ----
            emb = cpool.tile([VOCAB, C], F32R)
            nc.sync.dma_start(out=emb[:], in_=emb_d[:])
            w1e = []
            for k in range(K):
                w1ek = cpool.tile([VOCAB, C], F32R, name=f"w1e{k}")
                nc.sync.dma_start(out=w1ek[:], in_=w1e_d[k])
                w1e.append(w1ek)
            b1c = cpool.tile([128, NLEV * CCH], F32)
            nc.sync.dma_start(out=b1c[:], in_=b1_d[:])
            b2c = cpool.tile([128, NLEV * CCH], F32)
            nc.sync.dma_start(out=b2c[:], in_=b2_d[:])
            decT = []
            for ci in range(CCH):
                dct = cpool.tile([128, OUT], F32R, name=f"decT{ci}")
                nc.sync.dma_start(out=dct[:], in_=dec_d[ci])
                decT.append(dct)
            decb = cpool.tile([OUT, 1], F32)
            nc.sync.dma_start(out=decb[:], in_=db_d[:])

            ones_f = cpool.tile([1, 128], F32)
            nc.vector.memset(ones_f[:], 1.0)
            ones = cpool.tile([1, 128], F32R)
            nc.vector.tensor_copy(ones[:], ones_f[:])
            iota = cpool.tile([128, 1], F32)
            nc.gpsimd.iota(iota[:], pattern=[[0, 1]], base=0,
                           channel_multiplier=1,
                           allow_small_or_imprecise_dtypes=True)
            zpad = cpool.tile([128, PAD], F32)
            nc.vector.memset(zpad[:], 0.0)

            def zero_pads(t, nchunk=CCH):
                for cc in range(nchunk):
                    nc.vector.tensor_copy(t[:, cc * CW:cc * CW + PAD], zpad[:])

            for p in range(N_PASS):
                seqs = [p * SEQ_PER_PASS + s for s in range(SEQ_PER_PASS)]
                # hs[b] = current level-input state tile for sequence b
                hs = {}

                # ---- level 0: build one-hot, embedding, first level ----
                lev = 0
                d = 1
                ohs = {}
                for b in seqs:
                    xf = apool.tile([1, L], F32R, tag="xf", name=f"xf{b}")
                    nc.sync.dma_start(out=xf[:], in_=xf_d[b:b + 1, :])
                    oh = apool.tile([128, CW], F32R, tag="oh", name=f"oh{b}")
                    zero_pads(oh, 1)
                    for lc in range(LCH):
                        ps = pp.tile([128, 512], F32, tag="ps", name=f"psb{b}_{lc}")
                        nc.tensor.matmul(ps[:], ones[:],
                                         xf[:, lc * 512:(lc + 1) * 512],
                                         start=True, stop=True)
                        nc.vector.tensor_scalar(
                            out=oh[:, PAD + lc * 512:PAD + (lc + 1) * 512],
                            in0=ps[:], scalar1=iota[:], scalar2=None,
                            op0=ALU.is_equal)
                    ohs[b] = oh

                    # h0 = emb.T @ onehot  (residual input of level 0)
                    h0 = apool.tile([128, CCH * CW], F32R, tag="hs", bufs=3,
                                    name=f"h0_{b}")
                    zero_pads(h0)
                    for cc in range(CCH):
                        for lc in range(LCH):
                            ps = pp.tile([128, 512], F32, tag="ps", name=f"pse{b}_{cc}_{lc}")
                            nc.tensor.matmul(
                                ps[:], emb[:, cc * 128:(cc + 1) * 128],
                                oh[:, chunk(0, lc)], start=True, stop=True)
                            nc.vector.tensor_copy(h0[:, chunk(cc, lc)], ps[:])
                    hs[b] = h0

                # conv2 weights for level 0 are DMA'd after the one-hot /
                # embedding emission so the small input DMAs go out first
                w2_t = {}
                for k in range(K):
                    for ci in range(CCH):
                        w2_t[k, ci] = wpool.tile(
                            [128, C], F32R, tag=f"w2_{k}_{ci}", name=f"w2_{k}_{ci}")
                        nc.sync.dma_start(out=w2_t[k, ci][:],
                                          in_=w2t_d[lev, k, ci])

                for b in seqs:
                    oh = ohs[b]
                    # conv1 of level 0: contract over vocab with w1e
                    h1 = apool.tile([128, CCH * CW], F32R, tag="h1",
                                    name=f"h1_{b}_{lev}")
                    zero_pads(h1)
                    for cc in range(CCH):
                        for lc in range(LCH):
                            ps = pp.tile([128, 512], F32, tag="ps",
                                         name=f"ps1_{b}_{cc}_{lc}")
                            for k in range(K):
                                nc.tensor.matmul(
                                    ps[:],
                                    w1e[k][:, cc * 128:(cc + 1) * 128],
                                    oh[:, chunk(0, lc, (K - 1 - k) * d)],
                                    start=(k == 0), stop=(k == K - 1))
                            nc.scalar.activation(
                                h1[:, chunk(cc, lc)], ps[:], AF.Relu,
                                bias=b1c[:, lev * CCH + cc:lev * CCH + cc + 1],
                                scale=1.0)
                    _conv2_and_res(nc, tc, pp, apool, hs, b, lev, d, h1,
                                   w2_t, b2c, zero_pads, chunk, mybir)

                # ---- levels 1..3 ----
                for lev in range(1, NLEV):
                    d = 2 ** lev
                    w1_t = {}
                    for k in range(K):
                        for ci in range(CCH):
                            w1_t[k, ci] = wpool.tile(
                                [128, C], F32R, tag=f"w1_{k}_{ci}",
                                name=f"w1_{k}_{ci}_{lev}")
                            nc.sync.dma_start(out=w1_t[k, ci][:],
                                              in_=w1t_d[lev - 1, k, ci])
                    w2_t = {}
                    for k in range(K):
                        for ci in range(CCH):
                            w2_t[k, ci] = wpool.tile(
                                [128, C], F32R, tag=f"w2_{k}_{ci}",
                                name=f"w2_{k}_{ci}_{lev}")
                            nc.sync.dma_start(out=w2_t[k, ci][:],
                                              in_=w2t_d[lev, k, ci])

                    for b in seqs:
                        h = hs[b]
                        h1 = apool.tile([128, CCH * CW], F32R, tag="h1",
                                        name=f"h1_{b}_{lev}")
                        zero_pads(h1)
                        for cc in range(CCH):
                            for lc in range(LCH):
                                ps = pp.tile([128, 512], F32, tag="ps",
                                             name=f"ps1_{b}_{cc}_{lc}_{lev}")
                                first = True
                                for k in range(K):
                                    for ci in range(CCH):
                                        nc.tensor.matmul(
                                            ps[:],
                                            w1_t[k, ci][:, cc * 128:(cc + 1) * 128],
                                            h[:, chunk(ci, lc, (K - 1 - k) * d)],
                                            start=first,
                                            stop=(k == K - 1 and ci == CCH - 1))
                                        first = False
                                nc.scalar.activation(
                                    h1[:, chunk(cc, lc)], ps[:], AF.Relu,
                                    bias=b1c[:, lev * CCH + cc:lev * CCH + cc + 1],
                                    scale=1.0)
                        _conv2_and_res(nc, tc, pp, apool, hs, b, lev, d, h1,
                                       w2_t, b2c, zero_pads, chunk, mybir)

                # ---- decoder + mask ----
                for b in seqs:
                    h = hs[b]
                    mk = apool.tile([1, L], F32R, tag="mk", name=f"mk{b}")
                    nc.sync.dma_start(out=mk[:], in_=mkf_d[b:b + 1, :])
                    msk = apool.tile([OUT, L], F32, tag="msk", name=f"msk{b}")
                    ym = apool.tile([OUT, L], F32, tag="ym", name=f"ym{b}")
                    for lc in range(LCH):
                        psm = pp.tile([OUT, 512], F32, tag="ps", name=f"psm{b}_{lc}")
                        nc.tensor.matmul(psm[:], ones[:, 0:OUT],
                                         mk[:, lc * 512:(lc + 1) * 512],
                                         start=True, stop=True)
                        nc.vector.tensor_copy(
                            msk[:, lc * 512:(lc + 1) * 512], psm[:])
                        psd = pp.tile([OUT, 512], F32, tag="ps", name=f"psd{b}_{lc}")
                        for ci in range(CCH):
                            nc.tensor.matmul(
                                psd[:], decT[ci],
                                h[:, chunk(ci, lc)],
                                start=(ci == 0), stop=(ci == CCH - 1))
                        ysb = apool.tile([OUT, 512], F32, tag="ysb",
                                         name=f"ysb{b}_{lc}")
                        nc.scalar.activation(ysb[:], psd[:],
                                             AF.Identity,
                                             bias=decb[:], scale=1.0)
                        nc.vector.tensor_tensor(
                            out=ym[:, lc * 512:(lc + 1) * 512],
                            in0=ysb[:],
                            in1=msk[:, lc * 512:(lc + 1) * 512],
                            op=ALU.mult)
                    nc.sync.dma_start(out=y_d[b], in_=ym[:])

    nc.compile()
    return nc


def _conv2_and_res(nc, tc, pp, apool, hs, b, lev, d, h1, w2_t, b2c,
                   zero_pads, chunk, mybir):
    """conv2 of a level + residual; replaces hs[b] with the new state."""
    F32 = mybir.dt.float32
    F32R = mybir.dt.float32r
    AF = mybir.ActivationFunctionType
    ALU = mybir.AluOpType
    CCHl, LCHl = CCH, LCH
    h = hs[b]
    hn = apool.tile([128, CCH * CW], F32R, tag="hs", bufs=3,
                    name=f"hn_{b}_{lev}")
    zero_pads(hn)
    for cc in range(CCHl):
        for lc in range(LCHl):
            ps = pp.tile([128, 512], F32, tag="ps", name=f"ps2_{b}_{cc}_{lc}_{lev}")
            first = True
            for k in range(K):
                for ci in range(CCHl):
                    nc.tensor.matmul(
                        ps[:],
                        w2_t[k, ci][:, cc * 128:(cc + 1) * 128],
                        h1[:, chunk(ci, lc, (K - 1 - k) * d)],
                        start=first,
                        stop=(k == K - 1 and ci == CCHl - 1))
                    first = False
            h2 = apool.tile([128, 512], F32R, tag="h2",
                            name=f"h2_{b}_{cc}_{lc}_{lev}")
            nc.scalar.activation(
                h2[:], ps[:], AF.Relu,
                bias=b2c[:, lev * CCHl + cc:lev * CCHl + cc + 1], scale=1.0)
            if lev == 0:
                # h0 can be negative -> outer relu needed
                rt = apool.tile([128, 512], F32, tag="rtmp",
                                name=f"rt_{b}_{cc}_{lc}")
                nc.vector.tensor_tensor(out=rt[:], in0=h2[:],
                                        in1=h[:, chunk(cc, lc)], op=ALU.add)
                nc.scalar.activation(hn[:, chunk(cc, lc)], rt[:], AF.Relu)
            else:
                # both summands >= 0: outer relu is the identity
                nc.vector.tensor_tensor(out=hn[:, chunk(cc, lc)], in0=h2[:],
                                        in1=h[:, chunk(cc, lc)], op=ALU.add)
    hs[b] = hn


def _prep_inputs(x, mask, emb, w1, b1, w2, b2, dec_w, dec_b):
    """Host-side layout transforms; returns the per-core in_maps."""
    xf = x.astype(np.float32)
    mkf = mask.astype(np.float32)
    embf = np.asarray(emb, np.float32)
    w1f = np.asarray(w1, np.float32)
    w2f = np.asarray(w2, np.float32)
    # effective level-0 conv1 weights: contract over vocab instead of C
    # w1e[k][v, co] = sum_ci emb[v, ci] * w1[0][co, ci, k]
    w1e = np.stack([embf @ w1f[0, :, :, k].T for k in range(K)]).astype(np.float32)
    # [NLEV, K, Cin, Cout] chunked on Cin
    w1t = np.ascontiguousarray(w1f.transpose(0, 3, 2, 1))[1:].reshape(
        NLEV - 1, K, CCH, 128, C).astype(np.float32)
    w2t = np.ascontiguousarray(w2f.transpose(0, 3, 2, 1)).reshape(
        NLEV, K, CCH, 128, C).astype(np.float32)
    b1c = np.ascontiguousarray(
        np.asarray(b1, np.float32).reshape(NLEV, CCH, 128).transpose(2, 0, 1)
    ).reshape(128, NLEV * CCH)
    b2c = np.ascontiguousarray(
        np.asarray(b2, np.float32).reshape(NLEV, CCH, 128).transpose(2, 0, 1)
    ).reshape(128, NLEV * CCH)
    decT = np.ascontiguousarray(
        np.asarray(dec_w, np.float32).T.reshape(CCH, 128, OUT))
    decb = np.asarray(dec_b, np.float32).reshape(OUT, 1)

    in_maps = []
    for c in range(N_CORES):
        sl = slice(c * B_SHARD, (c + 1) * B_SHARD)
        in_maps.append(dict(
            xf=np.ascontiguousarray(xf[sl]),
            mkf=np.ascontiguousarray(mkf[sl]),
            emb=embf, w1e=w1e, w1t=w1t, w2t=w2t,
            b1c=b1c, b2c=b2c, decT=decT, decb=decb,
        ))
    return in_maps


def kernel(x, mask, emb, w1, b1, w2, b2, dec_w, dec_b):
    trace = bool(os.environ.get("BASS_TRACE"))
    if trace:
        _install_trace_shim()

    from concourse.bass_utils import run_bass_kernel_spmd

    if "nc" not in _CACHED:
        _CACHED["nc"] = _build()
    nc = _CACHED["nc"]

    in_maps = _prep_inputs(x, mask, emb, w1, b1, w2, b2, dec_w, dec_b)
    res = run_bass_kernel_spmd(nc, in_maps, list(range(N_CORES)),
                               trace=trace)
    _CACHED["last_result"] = res
    y = np.concatenate([res.results[c]["y"] for c in range(N_CORES)], axis=0)
    return np.ascontiguousarray(y.transpose(0, 2, 1))


# revision 11
# speedup vs baseline: 1.0318x; 1.0130x over previous
"""Trainium2 Bass kernel for the CPS-TCN model.

Model: embedding gather -> 4 TCN levels (2 dilated causal convs, K=2,
dilations 1,2,4,8, relu + residual) -> linear decoder [C=512 -> OUT=11]
-> multiplicative mask.  B=32, L=1024, C=512, VOCAB=128.

Sharding: data-parallel over B across 8 NeuronCores (4 sequences/core),
weights replicated.

Per-core implementation notes:
- The embedding gather is computed on the PE as emb.T @ onehot(x); the
  one-hot is built on-chip (iota + is_equal against a PE-broadcast of x).
- Conv taps are [128,128] x [128,512] matmuls accumulated in PSUM; the
  dilation shift is a free-dim slice into a left-zero-padded activation
  buffer (pad=8 >= max dilation).
- Level-0 conv1 contracts over the 128-entry vocab directly using
  host-precomputed effective weights (emb @ w1[0].T), saving 3/4 of the
  contraction chunks for that conv.
- All matmul operands are float32r (TF32-like single-pass PE mode,
  fp32 accumulate in PSUM).
- Levels >= 1 skip the outer residual relu: both summands are already
  non-negative.
- Inner loops are weight-major: one lhsT slice feeds the 4 psum groups
  (2 sequences x 2 L-chunks) back-to-back.
"""

import os
import sys
import types

sys.path.insert(0, "/opt/trn_rl_repo")

import numpy as np

B, L, C, K, NLEV, VOCAB, OUT = 32, 1024, 512, 2, 4, 128, 11
N_CORES = 8
B_SHARD = B // N_CORES          # 4 sequences per core
SEQ_PER_PASS = 2                # process 2 sequences per level sweep
N_PASS = B_SHARD // SEQ_PER_PASS
PAD = 8                         # left zero pad >= max dilation
CCH = C // 128                  # 4 channel chunks of 128
LCH = L // 512                  # 2 free-dim chunks of 512
CW = PAD + L                    # per-chunk width in the activation layout


def _install_trace_shim():
    """Register the axon NTFF profiling hook if tracing is requested."""
    import antenv

    if "antenv.axon_hooks" in sys.modules:
        return
    hooks_mod = types.ModuleType("antenv.axon_hooks")
    hooks_mod._hook = None

    def set_axon_ntff_profile_hook(h):
        hooks_mod._hook = h

    def get_axon_ntff_profile_hook():
        return hooks_mod._hook

    hooks_mod.set_axon_ntff_profile_hook = set_axon_ntff_profile_hook
    hooks_mod.get_axon_ntff_profile_hook = get_axon_ntff_profile_hook
    sys.modules["antenv.axon_hooks"] = hooks_mod
    antenv.axon_hooks = hooks_mod
    try:
        from trn_agent_boot.trn_boot import _ntff_profile_via_ctypes

        hook = _ntff_profile_via_ctypes("/opt/axon/libaxon_pjrt.so")
        set_axon_ntff_profile_hook(hook)
    except Exception:
        pass

    import concourse.bass_utils as bass_utils_mod

    bass_utils_mod.upload_artifacts = lambda tmpdir: "/tmp/no-upload"


_CACHED = {}


def _build():
    from concourse import bacc
    import concourse.mybir as mybir
    import concourse.tile as tile

    F32 = mybir.dt.float32
    F32R = mybir.dt.float32r
    AF = mybir.ActivationFunctionType
    ALU = mybir.AluOpType

    nc = bacc.Bacc("TRN2")

    # per sequence: row b holds x (first L) and mask (last L), both as f32
    xmk_d = nc.dram_tensor("xmk", [B_SHARD, 2 * L], F32R, kind="ExternalInput")
    emb_d = nc.dram_tensor("emb", [VOCAB, C], F32R, kind="ExternalInput")
    w1e_d = nc.dram_tensor("w1e", [VOCAB, K * C], F32R, kind="ExternalInput")
    w1t_d = nc.dram_tensor("w1t", [NLEV - 1, K, CCH, 128, C], F32R, kind="ExternalInput")
    w2t_d = nc.dram_tensor("w2t", [NLEV, K, CCH, 128, C], F32R, kind="ExternalInput")
    bb_d = nc.dram_tensor("bb", [128, 2 * NLEV * CCH], F32, kind="ExternalInput")
    dec_d = nc.dram_tensor("decT", [128, CCH * OUT], F32R, kind="ExternalInput")
    db_d = nc.dram_tensor("decb", [OUT, 1], F32, kind="ExternalInput")
    y_d = nc.dram_tensor("y", [B_SHARD, OUT, L], F32, kind="ExternalOutput")

    def chunk(off_cc, lc, shift=0):
        s = off_cc * CW + PAD + lc * 512 - shift
        return slice(s, s + 512)

    with tile.TileContext(nc) as tc:
        with tc.tile_pool(name="const", bufs=1) as cpool, \
             tc.tile_pool(name="wpool", bufs=2) as wpool, \
             tc.tile_pool(name="acts", bufs=2) as apool, \
             tc.tile_pool(name="psum", bufs=8, space="PSUM") as pp:

            # ---- constants (few, batched DMAs) ----
            emb = cpool.tile([VOCAB, C], F32R)
            nc.sync.dma_start(out=emb[:], in_=emb_d[:])
            w1e = cpool.tile([VOCAB, K * C], F32R)
            nc.sync.dma_start(out=w1e[:], in_=w1e_d[:])
            bb = cpool.tile([128, 2 * NLEV * CCH], F32)
            nc.sync.dma_start(out=bb[:], in_=bb_d[:])

            def bias1(lev, cc):
                return bb[:, lev * CCH + cc:lev * CCH + cc + 1]

            def bias2(lev, cc):
                o = NLEV * CCH + lev * CCH + cc
                return bb[:, o:o + 1]

            decT = cpool.tile([128, CCH * OUT], F32R)
            nc.sync.dma_start(out=decT[:], in_=dec_d[:])
            decb = cpool.tile([OUT, 1], F32)
            nc.sync.dma_start(out=decb[:], in_=db_d[:])

            ones_f = cpool.tile([1, 128], F32)
            nc.vector.memset(ones_f[:], 1.0)
            ones = cpool.tile([1, 128], F32R)
            nc.vector.tensor_copy(ones[:], ones_f[:])
            iota = cpool.tile([128, 1], F32)
            nc.gpsimd.iota(iota[:], pattern=[[0, 1]], base=0,
                           channel_multiplier=1,
                           allow_small_or_imprecise_dtypes=True)
            zpad = cpool.tile([128, PAD], F32)
            nc.vector.memset(zpad[:], 0.0)

            def zero_pads(t, nchunk=CCH):
                for cc in range(nchunk):
                    nc.vector.tensor_copy(t[:, cc * CW:cc * CW + PAD], zpad[:])

            def load_conv_w(dram_4d, lev, tagbase):
                # one DMA per conv: [128, K*CCH*C], (k, ci) chunks side by side
                t = wpool.tile([128, K * CCH * C], F32R, tag="w", bufs=3,
                               name=f"{tagbase}_{lev}")
                nc.sync.dma_start(
                    out=t[:].rearrange("p (k ci c) -> p k ci c", k=K, ci=CCH),
                    in_=dram_4d[lev].rearrange("k ci p c -> p k ci c"))
                return t

            def wsl(t, k, ci, cc):
                s = (k * CCH + ci) * C + cc * 128
                return t[:, s:s + 128]

            def conv(seqs, h_of, wt, n_ci, lev, d, drain):
                """Weight-major conv: psum groups (b, lc) accumulate over
                (k, ci); lhsT is reused across the 4 groups."""
                for cc in range(CCH):
                    ps = {}
                    for b in seqs:
                        for lc in range(LCH):
                            ps[b, lc] = pp.tile([128, 512], F32, tag="ps",
                                                name=f"ps_{b}_{cc}_{lc}_{lev}")
                    for k in range(K):
                        for ci in range(n_ci):
                            lhsT = wsl(wt, k, ci, cc) if n_ci > 1 else \
                                wt[:, k * C + cc * 128:k * C + (cc + 1) * 128]
                            for b in seqs:
                                for lc in range(LCH):
                                    nc.tensor.matmul(
                                        ps[b, lc], lhsT,
                                        h_of(b)[:, chunk(ci if n_ci > 1 else 0,
                                                         lc, (K - 1 - k) * d)],
                                        start=(k == 0 and ci == 0),
                                        stop=(k == K - 1 and ci == n_ci - 1))
                    for b in seqs:
                        for lc in range(LCH):
                            drain(b, cc, lc, ps[b, lc])

            for p in range(N_PASS):
                seqs = [p * SEQ_PER_PASS + s for s in range(SEQ_PER_PASS)]
                hs = {}

                # ---- level 0 ----
                lev, d = 0, 1
                ohs = {}
                xfs = {}
                for b in seqs:
                    xf = apool.tile([1, L], F32R, tag="xf", name=f"xf{b}")
                    nc.sync.dma_start(out=xf[:], in_=xmk_d[b:b + 1, 0:L])
                    xfs[b] = xf
                    oh = apool.tile([128, CW], F32R, tag="oh", name=f"oh{b}")
                    zero_pads(oh, 1)
                    ohs[b] = oh
                for b in seqs:
                    for lc in range(LCH):
                        psb = pp.tile([128, 512], F32, tag="ps",
                                      name=f"psb{b}_{lc}")
                        nc.tensor.matmul(psb[:], ones[:],
                                         xfs[b][:, lc * 512:(lc + 1) * 512],
                                         start=True, stop=True)
                        nc.vector.tensor_scalar(
                            out=ohs[b][:, PAD + lc * 512:PAD + (lc + 1) * 512],
                            in0=psb[:], scalar1=iota[:], scalar2=None,
                            op0=ALU.is_equal)

                # h0 = emb.T @ onehot (residual input of level 0)
                for b in seqs:
                    h0 = apool.tile([128, CCH * CW], F32R, tag="hs", bufs=4,
                                    name=f"h0_{b}")
                    zero_pads(h0)
                    hs[b] = h0
                for cc in range(CCH):
                    lhsT = emb[:, cc * 128:(cc + 1) * 128]
                    for b in seqs:
                        for lc in range(LCH):
                            pse = pp.tile([128, 512], F32, tag="ps",
                                          name=f"pse{b}_{cc}_{lc}")
                            nc.tensor.matmul(pse[:], lhsT, ohs[b][:, chunk(0, lc)],
                                             start=True, stop=True)
                            nc.vector.tensor_copy(hs[b][:, chunk(cc, lc)], pse[:])

                # weights arrive behind the small input DMAs
                w2_t = load_conv_w(w2t_d, 0, "w2")

                h1s = {b: apool.tile([128, CCH * CW], F32R, tag="h1",
                                     name=f"h1_{b}_0") for b in seqs}
                for b in seqs:
                    zero_pads(h1s[b])

                def drain1(b, cc, lc, ps, lev=lev):
                    nc.scalar.activation(h1s[b][:, chunk(cc, lc)], ps[:],
                                         AF.Relu, bias=bias1(lev, cc), scale=1.0)

                conv(seqs, lambda b: ohs[b], w1e, 1, lev, d, drain1)
                _conv2_res(nc, pp, apool, hs, seqs, lev, d, h1s, w2_t, bias2,
                           zero_pads, chunk, conv, mybir)

                # ---- levels 1..3 ----
                for lev in range(1, NLEV):
                    d = 2 ** lev
                    w1_t = load_conv_w(w1t_d, lev - 1, "w1")
                    w2_t = load_conv_w(w2t_d, lev, "w2")

                    h1s = {b: apool.tile([128, CCH * CW], F32R, tag="h1",
                                         name=f"h1_{b}_{lev}") for b in seqs}
                    for b in seqs:
                        zero_pads(h1s[b])

                    def drain1(b, cc, lc, ps, lev=lev):
                        nc.scalar.activation(h1s[b][:, chunk(cc, lc)], ps[:],
                                             AF.Relu, bias=bias1(lev, cc),
                                             scale=1.0)

                    conv(seqs, lambda b: hs[b], w1_t, CCH, lev, d, drain1)
                    _conv2_res(nc, pp, apool, hs, seqs, lev, d, h1s, w2_t,
                               bias2, zero_pads, chunk, conv, mybir)

                # ---- decoder + mask ----
                yms = {}
                for b in seqs:
                    mk = apool.tile([1, L], F32R, tag="mk", name=f"mk{b}")
                    nc.sync.dma_start(out=mk[:], in_=xmk_d[b:b + 1, L:2 * L])
                    ym = apool.tile([OUT, L], F32, tag="ym", name=f"ym{b}")
                    for lc in range(LCH):
                        psm = pp.tile([OUT, 512], F32, tag="ps",
                                      name=f"psm{b}_{lc}")
                        nc.tensor.matmul(psm[:], ones[:, 0:OUT],
                                         mk[:, lc * 512:(lc + 1) * 512],
                                         start=True, stop=True)
                        nc.vector.tensor_copy(ym[:, lc * 512:(lc + 1) * 512],
                                              psm[:])
                    yms[b] = ym
                psd = {}
                for b in seqs:
                    for lc in range(LCH):
                        psd[b, lc] = pp.tile([OUT, 512], F32, tag="ps",
                                             name=f"psd{b}_{lc}")
                for ci in range(CCH):
                    lhsT = decT[:, ci * OUT:(ci + 1) * OUT]
                    for b in seqs:
                        for lc in range(LCH):
                            nc.tensor.matmul(psd[b, lc], lhsT,
                                             hs[b][:, chunk(ci, lc)],
                                             start=(ci == 0),
                                             stop=(ci == CCH - 1))
                for b in seqs:
                    ym = yms[b]
                    for lc in range(LCH):
                        ysb = apool.tile([OUT, 512], F32, tag="ysb",
                                         name=f"ysb{b}_{lc}")
                        nc.scalar.activation(ysb[:], psd[b, lc], AF.Identity,
                                             bias=decb[:], scale=1.0)
                        nc.vector.tensor_tensor(
                            out=ym[:, lc * 512:(lc + 1) * 512], in0=ysb[:],
                            in1=ym[:, lc * 512:(lc + 1) * 512],
                            op=ALU.mult)
                    nc.sync.dma_start(out=y_d[b], in_=ym[:])

    nc.compile()
    return nc


def _conv2_res(nc, pp, apool, hs, seqs, lev, d, h1s, w2_t, bias2, zero_pads,
               chunk, conv, mybir):
    """conv2 + residual for a pair of sequences; updates hs[b]."""
    F32 = mybir.dt.float32
    F32R = mybir.dt.float32r
    AF = mybir.ActivationFunctionType
    ALU = mybir.AluOpType

    hns = {b: apool.tile([128, CCH * CW], F32R, tag="hs", bufs=4,
                         name=f"hn_{b}_{lev}") for b in seqs}
    for b in seqs:
        zero_pads(hns[b])

    def drain2(b, cc, lc, ps):
        h2 = apool.tile([128, 512], F32R, tag="h2", name=f"h2_{b}_{cc}_{lc}")
        nc.scalar.activation(h2[:], ps[:], AF.Relu, bias=bias2(lev, cc),
                             scale=1.0)
        if lev == 0:
            nc.vector.tensor_tensor(out=h2[:], in0=h2[:],
                                    in1=hs[b][:, chunk(cc, lc)], op=ALU.add)
            nc.scalar.activation(hns[b][:, chunk(cc, lc)], h2[:], AF.Relu)
        else:
            nc.vector.tensor_tensor(out=hns[b][:, chunk(cc, lc)], in0=h2[:],
                                    in1=hs[b][:, chunk(cc, lc)], op=ALU.add)

    conv(seqs, lambda b: h1s[b], w2_t, CCH, lev, d, drain2)
    for b in seqs:
        hs[b] = hns[b]


def _prep_inputs(x, mask, emb, w1, b1, w2, b2, dec_w, dec_b):
    """Host-side layout transforms; returns the per-core in_maps."""
    xf = x.astype(np.float32)
    mkf = mask.astype(np.float32)
    embf = np.asarray(emb, np.float32)
    w1f = np.asarray(w1, np.float32)
    w2f = np.asarray(w2, np.float32)
    # effective level-0 conv1 weights, packed [VOCAB, K*C]
    w1e = np.concatenate([embf @ w1f[0, :, :, k].T for k in range(K)],
                         axis=1).astype(np.float32)
    w1t = np.ascontiguousarray(w1f.transpose(0, 3, 2, 1))[1:].reshape(
        NLEV - 1, K, CCH, 128, C).astype(np.float32)
    w2t = np.ascontiguousarray(w2f.transpose(0, 3, 2, 1)).reshape(
        NLEV, K, CCH, 128, C).astype(np.float32)
    b1c = np.asarray(b1, np.float32).reshape(NLEV, CCH, 128).transpose(2, 0, 1)
    b2c = np.asarray(b2, np.float32).reshape(NLEV, CCH, 128).transpose(2, 0, 1)
    bb = np.ascontiguousarray(np.concatenate(
        [b1c.reshape(128, -1), b2c.reshape(128, -1)], axis=1))
    decT = np.ascontiguousarray(
        np.asarray(dec_w, np.float32).T.reshape(CCH, 128, OUT)
        .transpose(1, 0, 2).reshape(128, CCH * OUT))
    decb = np.asarray(dec_b, np.float32).reshape(OUT, 1)

    in_maps = []
    for c in range(N_CORES):
        sl = slice(c * B_SHARD, (c + 1) * B_SHARD)
        xmk = np.concatenate([xf[sl], mkf[sl]], axis=1)
        in_maps.append(dict(
            xmk=np.ascontiguousarray(xmk),
            emb=embf, w1e=w1e, w1t=w1t, w2t=w2t,
            bb=bb, decT=decT, decb=decb,
        ))
    return in_maps


def kernel(x, mask, emb, w1, b1, w2, b2, dec_w, dec_b):
    trace = bool(os.environ.get("BASS_TRACE"))
    if trace:
        _install_trace_shim()

    from concourse.bass_utils import run_bass_kernel_spmd

    if "nc" not in _CACHED:
        _CACHED["nc"] = _build()
    nc = _CACHED["nc"]

    in_maps = _prep_inputs(x, mask, emb, w1, b1, w2, b2, dec_w, dec_b)
    res = run_bass_kernel_spmd(nc, in_maps, list(range(N_CORES)),
                               trace=trace)
    _CACHED["last_result"] = res
    y = np.concatenate([res.results[c]["y"] for c in range(N_CORES)], axis=0)
    return np.ascontiguousarray(y.transpose(0, 2, 1))


# revision 13
# speedup vs baseline: 1.1026x; 1.0686x over previous
"""Trainium2 Bass kernel for the CPS-TCN model.

Model: embedding gather -> 4 TCN levels (2 dilated causal convs, K=2,
dilations 1,2,4,8, relu + residual) -> linear decoder [C=512 -> OUT=11]
-> multiplicative mask.  B=32, L=1024, C=512, VOCAB=128.

Sharding: data-parallel over B across 8 NeuronCores (4 sequences/core),
weights replicated.

Per-core implementation notes:
- The embedding gather is computed on the PE as emb.T @ onehot(x); the
  one-hot is built on-chip (iota + is_equal against a PE-broadcast of x).
- Conv taps are [128,128] x [128,512] matmuls accumulated in PSUM; the
  dilation shift is a free-dim slice into a left-zero-padded activation
  buffer (pad=8 >= max dilation).
- Level-0 conv1 contracts over the 128-entry vocab directly using
  host-precomputed effective weights (emb @ w1[0].T), saving 3/4 of the
  contraction chunks for that conv.
- All matmul operands are float32r (TF32-like single-pass PE mode,
  fp32 accumulate in PSUM).
- Levels >= 1 skip the outer residual relu: both summands are already
  non-negative.
- Inner loops are weight-major: one lhsT slice feeds the 4 psum groups
  (2 sequences x 2 L-chunks) back-to-back.
"""

import os
import sys
import types

sys.path.insert(0, "/opt/trn_rl_repo")

import numpy as np

B, L, C, K, NLEV, VOCAB, OUT = 32, 1024, 512, 2, 4, 128, 11
N_CORES = 8
B_SHARD = B // N_CORES          # 4 sequences per core
SEQ_PER_PASS = 2                # process 2 sequences per level sweep
N_PASS = B_SHARD // SEQ_PER_PASS
PAD = 8                         # left zero pad >= max dilation
CCH = C // 128                  # 4 channel chunks of 128
LCH = L // 512                  # 2 free-dim chunks of 512
CW = PAD + L                    # per-chunk width in the activation layout


def _install_trace_shim():
    """Register the axon NTFF profiling hook if tracing is requested."""
    import antenv

    if "antenv.axon_hooks" in sys.modules:
        return
    hooks_mod = types.ModuleType("antenv.axon_hooks")
    hooks_mod._hook = None

    def set_axon_ntff_profile_hook(h):
        hooks_mod._hook = h

    def get_axon_ntff_profile_hook():
        return hooks_mod._hook

    hooks_mod.set_axon_ntff_profile_hook = set_axon_ntff_profile_hook
    hooks_mod.get_axon_ntff_profile_hook = get_axon_ntff_profile_hook
    sys.modules["antenv.axon_hooks"] = hooks_mod
    antenv.axon_hooks = hooks_mod
    try:
        from trn_agent_boot.trn_boot import _ntff_profile_via_ctypes

        hook = _ntff_profile_via_ctypes("/opt/axon/libaxon_pjrt.so")
        set_axon_ntff_profile_hook(hook)
    except Exception:
        pass

    import concourse.bass_utils as bass_utils_mod

    bass_utils_mod.upload_artifacts = lambda tmpdir: "/tmp/no-upload"


_CACHED = {}


def _build():
    from concourse import bacc
    import concourse.mybir as mybir
    import concourse.tile as tile

    F32 = mybir.dt.float32
    F32R = (mybir.dt.bfloat16 if os.environ.get("MM_DT") == "bf16"
            else mybir.dt.float32r)
    AF = mybir.ActivationFunctionType
    ALU = mybir.AluOpType

    nc = bacc.Bacc("TRN2")

    # per sequence: row b holds x (first L) and mask (last L), both as f32
    xmk_d = nc.dram_tensor("xmk", [B_SHARD, 2 * L], F32R, kind="ExternalInput")
    emb_d = nc.dram_tensor("emb", [VOCAB, C], F32R, kind="ExternalInput")
    w1e_d = nc.dram_tensor("w1e", [VOCAB, K * C], F32R, kind="ExternalInput")
    w1t_d = nc.dram_tensor("w1t", [NLEV - 1, K, CCH, 128, C], F32R, kind="ExternalInput")
    w2t_d = nc.dram_tensor("w2t", [NLEV, K, CCH, 128, C], F32R, kind="ExternalInput")
    bb_d = nc.dram_tensor("bb", [128, 2 * NLEV * CCH], F32, kind="ExternalInput")
    dec_d = nc.dram_tensor("decT", [128, CCH * OUT], F32R, kind="ExternalInput")
    db_d = nc.dram_tensor("decb", [OUT, 1], F32, kind="ExternalInput")
    y_d = nc.dram_tensor("y", [B_SHARD, OUT, L], F32, kind="ExternalOutput")

    def chunk(off_cc, lc, shift=0):
        s = off_cc * CW + PAD + lc * 512 - shift
        return slice(s, s + 512)

    with tile.TileContext(nc) as tc:
        with tc.tile_pool(name="const", bufs=1) as cpool, \
             tc.tile_pool(name="wpool", bufs=2) as wpool, \
             tc.tile_pool(name="acts", bufs=2) as apool, \
             tc.tile_pool(name="psum", bufs=8, space="PSUM") as pp:

            # ---- constants (few, batched DMAs) ----
            emb = cpool.tile([VOCAB, C], F32R)
            nc.sync.dma_start(out=emb[:], in_=emb_d[:])
            w1e = cpool.tile([VOCAB, K * C], F32R)
            nc.sync.dma_start(out=w1e[:], in_=w1e_d[:])
            bb = cpool.tile([128, 2 * NLEV * CCH], F32)
            nc.sync.dma_start(out=bb[:], in_=bb_d[:])

            def bias1(lev, cc):
                return bb[:, lev * CCH + cc:lev * CCH + cc + 1]

            def bias2(lev, cc):
                o = NLEV * CCH + lev * CCH + cc
                return bb[:, o:o + 1]

            decT = cpool.tile([128, CCH * OUT], F32R)
            nc.sync.dma_start(out=decT[:], in_=dec_d[:])
            decb = cpool.tile([OUT, 1], F32)
            nc.sync.dma_start(out=decb[:], in_=db_d[:])

            ones_f = cpool.tile([1, 128], F32)
            nc.vector.memset(ones_f[:], 1.0)
            ones = cpool.tile([1, 128], F32R)
            nc.vector.tensor_copy(ones[:], ones_f[:])
            iota = cpool.tile([128, 1], F32)
            nc.gpsimd.iota(iota[:], pattern=[[0, 1]], base=0,
                           channel_multiplier=1,
                           allow_small_or_imprecise_dtypes=True)
            zpad = cpool.tile([128, PAD], F32)
            nc.vector.memset(zpad[:], 0.0)

            def zero_pads(t, nchunk=CCH):
                for cc in range(nchunk):
                    nc.vector.tensor_copy(t[:, cc * CW:cc * CW + PAD], zpad[:])

            def load_conv_w(dram_4d, lev, tagbase):
                # one DMA per conv: [128, K*CCH*C], (k, ci) chunks side by side
                t = wpool.tile([128, K * CCH * C], F32R, tag="w", bufs=3,
                               name=f"{tagbase}_{lev}")
                nc.sync.dma_start(
                    out=t[:].rearrange("p (k ci c) -> p k ci c", k=K, ci=CCH),
                    in_=dram_4d[lev].rearrange("k ci p c -> p k ci c"))
                return t

            def wsl(t, k, ci, cc):
                s = (k * CCH + ci) * C + cc * 128
                return t[:, s:s + 128]

            def conv(seqs, h_of, wt, n_ci, lev, d, drain):
                """Weight-major conv: psum groups (b, lc) accumulate over
                (k, ci); lhsT is reused across the 4 groups."""
                for cc in range(CCH):
                    ps = {}
                    for b in seqs:
                        for lc in range(LCH):
                            ps[b, lc] = pp.tile([128, 512], F32, tag="ps",
                                                name=f"ps_{b}_{cc}_{lc}_{lev}")
                    for k in range(K):
                        for ci in range(n_ci):
                            lhsT = wsl(wt, k, ci, cc) if n_ci > 1 else \
                                wt[:, k * C + cc * 128:k * C + (cc + 1) * 128]
                            for b in seqs:
                                for lc in range(LCH):
                                    nc.tensor.matmul(
                                        ps[b, lc], lhsT,
                                        h_of(b)[:, chunk(ci if n_ci > 1 else 0,
                                                         lc, (K - 1 - k) * d)],
                                        start=(k == 0 and ci == 0),
                                        stop=(k == K - 1 and ci == n_ci - 1))
                    for b in seqs:
                        for lc in range(LCH):
                            drain(b, cc, lc, ps[b, lc])

            for p in range(N_PASS):
                seqs = [p * SEQ_PER_PASS + s for s in range(SEQ_PER_PASS)]
                hs = {}

                # ---- level 0 ----
                lev, d = 0, 1
                ohs = {}
                xfs = {}
                for b in seqs:
                    xf = apool.tile([1, L], F32R, tag="xf", name=f"xf{b}")
                    nc.sync.dma_start(out=xf[:], in_=xmk_d[b:b + 1, 0:L])
                    xfs[b] = xf
                    oh = apool.tile([128, CW], F32R, tag="oh", name=f"oh{b}")
                    zero_pads(oh, 1)
                    ohs[b] = oh
                for b in seqs:
                    for lc in range(LCH):
                        psb = pp.tile([128, 512], F32, tag="ps",
                                      name=f"psb{b}_{lc}")
                        nc.tensor.matmul(psb[:], ones[:],
                                         xfs[b][:, lc * 512:(lc + 1) * 512],
                                         start=True, stop=True)
                        nc.vector.tensor_scalar(
                            out=ohs[b][:, PAD + lc * 512:PAD + (lc + 1) * 512],
                            in0=psb[:], scalar1=iota[:], scalar2=None,
                            op0=ALU.is_equal)

                # h0 = emb.T @ onehot (residual input of level 0)
                for b in seqs:
                    h0 = apool.tile([128, CCH * CW], F32R, tag="hs", bufs=4,
                                    name=f"h0_{b}")
                    zero_pads(h0)
                    hs[b] = h0
                for cc in range(CCH):
                    lhsT = emb[:, cc * 128:(cc + 1) * 128]
                    for b in seqs:
                        for lc in range(LCH):
                            pse = pp.tile([128, 512], F32, tag="ps",
                                          name=f"pse{b}_{cc}_{lc}")
                            nc.tensor.matmul(pse[:], lhsT, ohs[b][:, chunk(0, lc)],
                                             start=True, stop=True)
                            nc.vector.tensor_copy(hs[b][:, chunk(cc, lc)], pse[:])

                # weights arrive behind the small input DMAs
                w2_t = load_conv_w(w2t_d, 0, "w2")

                h1s = {b: apool.tile([128, CCH * CW], F32R, tag="h1",
                                     name=f"h1_{b}_0") for b in seqs}
                for b in seqs:
                    zero_pads(h1s[b])

                def drain1(b, cc, lc, ps, lev=lev):
                    nc.scalar.activation(h1s[b][:, chunk(cc, lc)], ps[:],
                                         AF.Relu, bias=bias1(lev, cc), scale=1.0)

                conv(seqs, lambda b: ohs[b], w1e, 1, lev, d, drain1)
                _conv2_res(nc, pp, apool, hs, seqs, lev, d, h1s, w2_t, bias2,
                           zero_pads, chunk, conv, mybir)

                # ---- levels 1..3 ----
                for lev in range(1, NLEV):
                    d = 2 ** lev
                    w1_t = load_conv_w(w1t_d, lev - 1, "w1")
                    w2_t = load_conv_w(w2t_d, lev, "w2")

                    h1s = {b: apool.tile([128, CCH * CW], F32R, tag="h1",
                                         name=f"h1_{b}_{lev}") for b in seqs}
                    for b in seqs:
                        zero_pads(h1s[b])

                    def drain1(b, cc, lc, ps, lev=lev):
                        nc.scalar.activation(h1s[b][:, chunk(cc, lc)], ps[:],
                                             AF.Relu, bias=bias1(lev, cc),
                                             scale=1.0)

                    conv(seqs, lambda b: hs[b], w1_t, CCH, lev, d, drain1)
                    _conv2_res(nc, pp, apool, hs, seqs, lev, d, h1s, w2_t,
                               bias2, zero_pads, chunk, conv, mybir)

                # ---- decoder + mask ----
                yms = {}
                for b in seqs:
                    mk = apool.tile([1, L], F32R, tag="mk", name=f"mk{b}")
                    nc.sync.dma_start(out=mk[:], in_=xmk_d[b:b + 1, L:2 * L])
                    ym = apool.tile([OUT, L], F32, tag="ym", name=f"ym{b}")
                    for lc in range(LCH):
                        psm = pp.tile([OUT, 512], F32, tag="ps",
                                      name=f"psm{b}_{lc}")
                        nc.tensor.matmul(psm[:], ones[:, 0:OUT],
                                         mk[:, lc * 512:(lc + 1) * 512],
                                         start=True, stop=True)
                        nc.vector.tensor_copy(ym[:, lc * 512:(lc + 1) * 512],
                                              psm[:])
                    yms[b] = ym
                psd = {}
                for b in seqs:
                    for lc in range(LCH):
                        psd[b, lc] = pp.tile([OUT, 512], F32, tag="ps",
                                             name=f"psd{b}_{lc}")
                for ci in range(CCH):
                    lhsT = decT[:, ci * OUT:(ci + 1) * OUT]
                    for b in seqs:
                        for lc in range(LCH):
                            nc.tensor.matmul(psd[b, lc], lhsT,
                                             hs[b][:, chunk(ci, lc)],
                                             start=(ci == 0),
                                             stop=(ci == CCH - 1))
                for b in seqs:
                    ym = yms[b]
                    for lc in range(LCH):
                        ysb = apool.tile([OUT, 512], F32, tag="ysb",
                                         name=f"ysb{b}_{lc}")
                        nc.scalar.activation(ysb[:], psd[b, lc], AF.Identity,
                                             bias=decb[:], scale=1.0)
                        nc.vector.tensor_tensor(
                            out=ym[:, lc * 512:(lc + 1) * 512], in0=ysb[:],
                            in1=ym[:, lc * 512:(lc + 1) * 512],
                            op=ALU.mult)
                    nc.sync.dma_start(out=y_d[b], in_=ym[:])

    nc.compile()
    return nc


def _conv2_res(nc, pp, apool, hs, seqs, lev, d, h1s, w2_t, bias2, zero_pads,
               chunk, conv, mybir):
    """conv2 + residual for a pair of sequences; updates hs[b]."""
    F32 = mybir.dt.float32
    F32R = (mybir.dt.bfloat16 if os.environ.get("MM_DT") == "bf16"
            else mybir.dt.float32r)
    AF = mybir.ActivationFunctionType
    ALU = mybir.AluOpType

    hns = {b: apool.tile([128, CCH * CW], F32R, tag="hs", bufs=4,
                         name=f"hn_{b}_{lev}") for b in seqs}
    for b in seqs:
        zero_pads(hns[b])

    def drain2(b, cc, lc, ps):
        h2 = apool.tile([128, 512], F32R, tag="h2", name=f"h2_{b}_{cc}_{lc}")
        nc.scalar.activation(h2[:], ps[:], AF.Relu, bias=bias2(lev, cc),
                             scale=1.0)
        if lev == 0:
            nc.vector.tensor_tensor(out=h2[:], in0=h2[:],
                                    in1=hs[b][:, chunk(cc, lc)], op=ALU.add)
            nc.scalar.activation(hns[b][:, chunk(cc, lc)], h2[:], AF.Relu)
        else:
            nc.vector.tensor_tensor(out=hns[b][:, chunk(cc, lc)], in0=h2[:],
                                    in1=hs[b][:, chunk(cc, lc)], op=ALU.add)

    conv(seqs, lambda b: h1s[b], w2_t, CCH, lev, d, drain2)
    for b in seqs:
        hs[b] = hns[b]


def _prep_inputs(x, mask, emb, w1, b1, w2, b2, dec_w, dec_b):
    """Host-side layout transforms; returns the per-core in_maps."""
    if os.environ.get("MM_DT") == "bf16":
        import ml_dtypes
        mmdt = ml_dtypes.bfloat16
    else:
        mmdt = np.float32
    xf = x.astype(np.float32)
    mkf = mask.astype(np.float32)
    embf = np.asarray(emb, np.float32)
    w1f = np.asarray(w1, np.float32)
    w2f = np.asarray(w2, np.float32)
    # effective level-0 conv1 weights, packed [VOCAB, K*C]
    w1e = np.concatenate([embf @ w1f[0, :, :, k].T for k in range(K)],
                         axis=1).astype(np.float32)
    w1t = np.ascontiguousarray(w1f.transpose(0, 3, 2, 1))[1:].reshape(
        NLEV - 1, K, CCH, 128, C).astype(np.float32)
    w2t = np.ascontiguousarray(w2f.transpose(0, 3, 2, 1)).reshape(
        NLEV, K, CCH, 128, C).astype(np.float32)
    b1c = np.asarray(b1, np.float32).reshape(NLEV, CCH, 128).transpose(2, 0, 1)
    b2c = np.asarray(b2, np.float32).reshape(NLEV, CCH, 128).transpose(2, 0, 1)
    bb = np.ascontiguousarray(np.concatenate(
        [b1c.reshape(128, -1), b2c.reshape(128, -1)], axis=1))
    decT = np.ascontiguousarray(
        np.asarray(dec_w, np.float32).T.reshape(CCH, 128, OUT)
        .transpose(1, 0, 2).reshape(128, CCH * OUT))
    decb = np.asarray(dec_b, np.float32).reshape(OUT, 1)

    in_maps = []
    for c in range(N_CORES):
        sl = slice(c * B_SHARD, (c + 1) * B_SHARD)
        xmk = np.concatenate([xf[sl], mkf[sl]], axis=1)
        in_maps.append(dict(
            xmk=np.ascontiguousarray(xmk).astype(mmdt),
            emb=embf.astype(mmdt), w1e=w1e.astype(mmdt),
            w1t=w1t.astype(mmdt), w2t=w2t.astype(mmdt),
            bb=bb, decT=decT.astype(mmdt), decb=decb,
        ))
    return in_maps


def kernel(x, mask, emb, w1, b1, w2, b2, dec_w, dec_b):
    trace = bool(os.environ.get("BASS_TRACE"))
    if trace:
        _install_trace_shim()

    from concourse.bass_utils import run_bass_kernel_spmd

    if "nc" not in _CACHED:
        _CACHED["nc"] = _build()
    nc = _CACHED["nc"]

    in_maps = _prep_inputs(x, mask, emb, w1, b1, w2, b2, dec_w, dec_b)
    res = run_bass_kernel_spmd(nc, in_maps, list(range(N_CORES)),
                               trace=trace)
    _CACHED["last_result"] = res
    y = np.concatenate([res.results[c]["y"] for c in range(N_CORES)], axis=0)
    return np.ascontiguousarray(y.transpose(0, 2, 1))
